# revision 4
# baseline (speedup 1.0000x reference)
"""GQA attention kernel for 8 trn2 NeuronCores.

Sharding: B(2) x KV-groups(4) = 8 cores. Core c handles batch b=c//4 and
kv-head g=c%4 with its 4 matching q-heads {g, g+4, g+8, g+12}. Each core
computes its partial output projection y_b_partial = attn_slice @ wo_rows;
the host sums the 4 partials per batch (row-parallel wo unshard).

Device layout notes:
- hd channels of q/k are host-permuted to de-interleaved (x0s then x1s)
  order so RoPE halves are contiguous partition blocks; the permutation
  cancels inside the q.k contraction.
- Scores are built transposed (j=q-index on partitions, i=k-index free) so
  exp'd scores serve directly as AV-matmul lhsT and the attention output
  lands transposed, ready to be the output-projection lhsT. Causal
  structure skips fully-masked tiles; diagonal-band tiles get their
  additive mask accumulated into PSUM via an identity-lhsT matmul.
- float32r (full-rate fp32 PE mode) everywhere on matmul inputs.
"""
import numpy as np

B, T, C = 2, 1024, 2048
NH, NKV, HD = 16, 4, 128
NREP = NH // NKV
NC_ = 8
NCC = C // 128          # 16 contraction chunks
EXP_BIAS = -40.0        # constant shift inside exp; cancels in normalization

_prog_cache = {}


def _build_program():
    import concourse.mybir as mybir
    from concourse import bacc
    from concourse.tile import TileContext

    f32 = mybir.dt.float32
    f32r = mybir.dt.float32r
    AF = mybir.ActivationFunctionType
    OP = mybir.AluOpType

    nc = bacc.Bacc("TRN2", target_bir_lowering=False, debug=False,
                   num_devices=NC_)

    xt_d = nc.dram_tensor("xt", [C, T], f32r, kind="ExternalInput").ap()
    wq_d = nc.dram_tensor("wq", [C, 512], f32r, kind="ExternalInput").ap()
    wk_d = nc.dram_tensor("wk", [C, 128], f32r, kind="ExternalInput").ap()
    wv_d = nc.dram_tensor("wv", [C, 128], f32r, kind="ExternalInput").ap()
    wo_d = nc.dram_tensor("wo", [512, C], f32r, kind="ExternalInput").ap()
    cos_d = nc.dram_tensor("cosT", [128, T], f32, kind="ExternalInput").ap()
    sin_d = nc.dram_tensor("sinT", [128, T], f32, kind="ExternalInput").ap()
    msk_d = nc.dram_tensor("masks", [128, 4 * 512], f32r, kind="ExternalInput").ap()
    idn_d = nc.dram_tensor("idn", [128, 128], f32r, kind="ExternalInput").ap()
    ones_d = nc.dram_tensor("ones", [128, 130], f32r, kind="ExternalInput").ap()
    y_d = nc.dram_tensor("y", [T, C], f32, kind="ExternalOutput").ap()

    with TileContext(nc) as tc:
        with tc.tile_pool(name="persist", bufs=1) as pp:
            cosT = pp.tile([128, T], f32, tag="cosT")
            sinT = pp.tile([128, T], f32, tag="sinT")
            masks = pp.tile([128, 4 * 512], f32r, tag="masks")
            idn = pp.tile([128, 128], f32r, tag="idn")
            ones = pp.tile([128, 130], f32r, tag="ones")
            bias_t = pp.tile([128, 1], f32, tag="bias")
            nc.sync.dma_start(out=cosT, in_=cos_d[:])
            nc.sync.dma_start(out=sinT, in_=sin_d[:])
            nc.sync.dma_start(out=masks, in_=msk_d[:])
            nc.sync.dma_start(out=idn, in_=idn_d[:])
            nc.sync.dma_start(out=ones, in_=ones_d[:])
            nc.vector.memset(bias_t, EXP_BIAS)

            qT = [pp.tile([128, T], f32r, tag=f"qT{h}", name=f"qT{h}") for h in range(4)]
            kT = pp.tile([128, T], f32r, tag="kT")
            v = [pp.tile([128, 128], f32r, tag=f"v{jc}", name=f"v{jc}") for jc in range(8)]
            attnT = [pp.tile([128, T], f32r, tag=f"attnT{h}", name=f"attnT{h}") for h in range(4)]

            # ---------------- Phase 1: projections + RoPE ----------------
            with tc.tile_pool(name="ph1", bufs=1) as wp, \
                 tc.tile_pool(name="ph1work", bufs=4) as wk_pool, \
                 tc.tile_pool(name="ps1", bufs=4, space="PSUM") as ps1:
                xt_t, wq_t, wk_t, wv_t = [], [], [], []
                for cc in range(NCC):
                    xt = wp.tile([128, T], f32r, tag=f"xt{cc}")
                    nc.sync.dma_start(out=xt, in_=xt_d[cc * 128:(cc + 1) * 128, :])
                    xt_t.append(xt)
                    wqt = wp.tile([128, 512], f32r, tag=f"wq{cc}")
                    nc.sync.dma_start(out=wqt, in_=wq_d[cc * 128:(cc + 1) * 128, :])
                    wq_t.append(wqt)
                    wkt = wp.tile([128, 128], f32r, tag=f"wk{cc}")
                    nc.sync.dma_start(out=wkt, in_=wk_d[cc * 128:(cc + 1) * 128, :])
                    wk_t.append(wkt)
                    wvt = wp.tile([128, 128], f32r, tag=f"wv{cc}")
                    nc.sync.dma_start(out=wvt, in_=wv_d[cc * 128:(cc + 1) * 128, :])
                    wv_t.append(wvt)

                def rope(dst, ps, t2):
                    """dst[:, t2*512:+512] = rot(ps) using cosT/sinT slices."""
                    sl = slice(t2 * 512, (t2 + 1) * 512)
                    swp = wk_pool.tile([128, 512], f32, tag="swp")
                    nc.vector.tensor_copy(out=swp[0:64], in_=ps[64:128])
                    nc.vector.tensor_copy(out=swp[64:128], in_=ps[0:64])
                    t1 = wk_pool.tile([128, 512], f32, tag="t1")
                    nc.vector.tensor_tensor(out=t1, in0=ps, in1=cosT[:, sl],
                                            op=OP.mult)
                    t2b = wk_pool.tile([128, 512], f32, tag="t2b")
                    nc.vector.tensor_tensor(out=t2b, in0=swp, in1=sinT[:, sl],
                                            op=OP.mult)
                    nc.vector.tensor_tensor(out=dst[:, sl], in0=t1, in1=t2b,
                                            op=OP.add)

                for h in range(4):
                    for t2 in range(2):
                        ps = ps1.tile([128, 512], f32, tag="proj")
                        for cc in range(NCC):
                            nc.tensor.matmul(
                                out=ps,
                                lhsT=wq_t[cc][:, h * 128:(h + 1) * 128],
                                rhs=xt_t[cc][:, t2 * 512:(t2 + 1) * 512],
                                start=(cc == 0), stop=(cc == NCC - 1))
                        rope(qT[h], ps, t2)
                for t2 in range(2):
                    ps = ps1.tile([128, 512], f32, tag="proj")
                    for cc in range(NCC):
                        nc.tensor.matmul(out=ps, lhsT=wk_t[cc],
                                         rhs=xt_t[cc][:, t2 * 512:(t2 + 1) * 512],
                                         start=(cc == 0), stop=(cc == NCC - 1))
                    rope(kT, ps, t2)
                # vT then PE-transpose to v (T on partitions)
                for t2 in range(2):
                    ps = ps1.tile([128, 512], f32, tag="proj")
                    for cc in range(NCC):
                        nc.tensor.matmul(out=ps, lhsT=wv_t[cc],
                                         rhs=xt_t[cc][:, t2 * 512:(t2 + 1) * 512],
                                         start=(cc == 0), stop=(cc == NCC - 1))
                    vts = wk_pool.tile([128, 512], f32r, tag="vts")
                    nc.scalar.copy(out=vts, in_=ps)
                    for q4 in range(4):
                        jc = t2 * 4 + q4
                        pst = ps1.tile([128, 128], f32r, tag="vtr")
                        nc.tensor.transpose(pst, vts[:, q4 * 128:(q4 + 1) * 128],
                                            idn)
                        nc.scalar.copy(out=v[jc], in_=pst)

            # ---------------- Phase 2: attention per head ----------------
            with tc.tile_pool(name="att", bufs=1) as ap_, \
                 tc.tile_pool(name="attw", bufs=3) as aw, \
                 tc.tile_pool(name="ps2o", bufs=2, space="PSUM") as ps2o, \
                 tc.tile_pool(name="ps2r", bufs=1, space="PSUM") as ps2r, \
                 tc.tile_pool(name="ps2b", bufs=1, space="PSUM") as ps2b, \
                 tc.tile_pool(name="ps2s", bufs=3, space="PSUM") as ps2s:
                for h in range(4):
                    E = {}
                    for jc in range(8):
                        for ic in ([0, 1] if jc < 4 else [1]):
                            o = 128 * jc - 512 * ic
                            psS = ps2s.tile([128, 512], f32, tag="S")
                            first = True
                            if 0 <= o <= 384:
                                m = o // 128
                                nc.tensor.matmul(
                                    out=psS, lhsT=idn,
                                    rhs=masks[:, m * 512:(m + 1) * 512],
                                    start=True, stop=False)
                                first = False
                            nc.tensor.matmul(
                                out=psS,
                                lhsT=qT[h][:, jc * 128:(jc + 1) * 128],
                                rhs=kT[:, ic * 512:(ic + 1) * 512],
                                start=first, stop=True)
                            e = ap_.tile([128, 512], f32r, tag=f"E{jc}_{ic}")
                            nc.scalar.activation(out=e, in_=psS, func=AF.Exp,
                                                 bias=bias_t, scale=1.0)
                            E[(jc, ic)] = e
                    # row sums r (1, i) and reciprocal
                    rec = aw.tile([1, T], f32r, tag="rec")
                    for ic in range(2):
                        live = range(4 * ic + 4)
                        psr = ps2r.tile([1, 512], f32, tag="r")
                        for n_, jc in enumerate(live):
                            nc.tensor.matmul(out=psr, lhsT=ones[:, 0:1],
                                             rhs=E[(jc, ic)],
                                             start=(n_ == 0),
                                             stop=(n_ == len(live) - 1))
                        rs = aw.tile([1, 512], f32, tag="rs")
                        nc.vector.reciprocal(out=rs, in_=psr)
                        nc.vector.tensor_copy(
                            out=rec[:, ic * 512:(ic + 1) * 512], in_=rs)
                    # AV: O^T accumulates over jc; bcast recip; normalize
                    for ic in range(2):
                        live = list(range(4 * ic + 4))
                        psO = ps2o.tile([128, 512], f32, tag="O")
                        for n_, jc in enumerate(live):
                            nc.tensor.matmul(out=psO, lhsT=v[jc],
                                             rhs=E[(jc, ic)],
                                             start=(n_ == 0),
                                             stop=(n_ == len(live) - 1))
                        psB = ps2b.tile([128, 512], f32, tag="bc")
                        nc.tensor.matmul(out=psB, lhsT=ones[0:1, 0:128],
                                         rhs=rec[:, ic * 512:(ic + 1) * 512],
                                         start=True, stop=True)
                        bcs = aw.tile([128, 512], f32, tag="bcs")
                        nc.scalar.copy(out=bcs, in_=psB)
                        nc.vector.tensor_tensor(
                            out=attnT[h][:, ic * 512:(ic + 1) * 512],
                            in0=psO, in1=bcs, op=OP.mult)

            # ---------------- Phase 3: output projection ----------------
            with tc.tile_pool(name="ph3", bufs=1) as op_, \
                 tc.tile_pool(name="ph3w", bufs=4) as ow, \
                 tc.tile_pool(name="ps3", bufs=4, space="PSUM") as ps3:
                wo_t = []
                for cc in range(4):
                    wot = op_.tile([128, C], f32r, tag=f"wo{cc}")
                    nc.sync.dma_start(out=wot, in_=wo_d[cc * 128:(cc + 1) * 128, :])
                    wo_t.append(wot)
                for tcb in range(8):
                    for ncol in range(4):
                        psy = ps3.tile([128, 512], f32, tag="y")
                        for cc in range(4):
                            nc.tensor.matmul(
                                out=psy,
                                lhsT=attnT[cc][:, tcb * 128:(tcb + 1) * 128],
                                rhs=wo_t[cc][:, ncol * 512:(ncol + 1) * 512],
                                start=(cc == 0), stop=(cc == 3))
                        ys = ow.tile([128, 512], f32, tag="ys")
                        if (tcb + ncol) % 2 == 0:
                            nc.scalar.copy(out=ys, in_=psy)
                        else:
                            nc.vector.tensor_copy(out=ys, in_=psy)
                        nc.sync.dma_start(
                            out=y_d[tcb * 128:(tcb + 1) * 128,
                                    ncol * 512:(ncol + 1) * 512],
                            in_=ys)

    nc.finalize()
    return nc


def _host_prep(x, angles, wq, wk, wv, wo):
    perm = np.concatenate([np.arange(0, HD, 2), np.arange(1, HD, 2)])
    cosA = np.cos(angles).astype(np.float32)   # (T, 64)
    sinA = np.sin(angles).astype(np.float32)
    cosT = np.empty((128, T), np.float32)
    sinT = np.empty((128, T), np.float32)
    cosT[0:64] = cosA.T
    cosT[64:128] = cosA.T
    sinT[0:64] = -sinA.T
    sinT[64:128] = sinA.T

    masks = np.zeros((128, 4 * 512), np.float32)
    p = np.arange(128)[:, None]
    f = np.arange(512)[None, :]
    for m in range(4):
        masks[:, m * 512:(m + 1) * 512] = np.where(
            f < p + m * 128, -1e30, 0.0).astype(np.float32)
    idn = np.eye(128, dtype=np.float32)
    ones = np.ones((128, 130), np.float32)

    in_maps = []
    for c in range(NC_):
        b, g = c // 4, c % 4
        heads = [g + NKV * r for r in range(NREP)]
        wq_c = np.concatenate(
            [wq[:, h * HD:(h + 1) * HD][:, perm] for h in heads], axis=1)
        wk_c = wk[:, g * HD:(g + 1) * HD][:, perm]
        wv_c = wv[:, g * HD:(g + 1) * HD]
        wo_c = np.concatenate(
            [wo[h * HD:(h + 1) * HD, :] for h in heads], axis=0)
        in_maps.append({
            "xt": np.ascontiguousarray(x[b].T).astype(np.float32),
            "wq": np.ascontiguousarray(wq_c, dtype=np.float32),
            "wk": np.ascontiguousarray(wk_c, dtype=np.float32),
            "wv": np.ascontiguousarray(wv_c, dtype=np.float32),
            "wo": np.ascontiguousarray(wo_c, dtype=np.float32),
            "cosT": cosT, "sinT": sinT, "masks": masks,
            "idn": idn, "ones": ones,
        })
    return in_maps


def kernel(x, angles, wq, wk, wv, wo, _trace=False):
    from concourse.bass_utils import run_bass_kernel_spmd
    if "nc" not in _prog_cache:
        _prog_cache["nc"] = _build_program()
    nc = _prog_cache["nc"]
    in_maps = _host_prep(np.asarray(x), np.asarray(angles), np.asarray(wq),
                         np.asarray(wk), np.asarray(wv), np.asarray(wo))
    res = run_bass_kernel_spmd(nc, in_maps, list(range(NC_)), trace=_trace)
    _prog_cache["last_res"] = res
    out = np.empty((B, T, C), np.float32)
    for b in range(B):
        acc = np.zeros((T, C), np.float64)
        for g in range(4):
            acc += res.results[4 * b + g]["y"]
        out[b] = acc.astype(np.float32)
    return out
